# revision 9
# baseline (speedup 1.0000x reference)
"""Trainium2 Bass kernel for nn_BestHits: out = bh * bh.T where
bh = blockwise-softmax(mask_diag(similarities) / TAU) over 256-wide column groups.

Strategy: out is symmetric, so only the 136 upper-incl-diagonal 512x512
block-pairs are computed (17 per core on 8 cores); out[J,I] = out[I,J].T is
mirrored on the host.

fp16 end-to-end to halve HBM traffic (memory-bound problem): the host
subtracts the per-(row, 256-group) max before quantizing to fp16 (softmax is
shift-invariant, and the shift makes fp16 rounding error negligible: the
dominant softmax terms sit near 0 where fp16 absolute error is tiny).
Outputs are stored fp16 and widened to f32 on the host.

Per pair (I, J) the device sees A = Y[I, J]-block and W-input = Y[J, I].T
(transposed on host during staging, so the reciprocal product is purely
elementwise on device -- no PE transpose):
  out[I,J][r, c] = za[r, c]/sa[r, g(c)] * W[r, c]/sB[g(r), c]
with za = exp(10*A), W = exp(10*B.T),
  sa[r, g]  = row-group sums of za        (free-dim reduce: DVE/Pool),
  sB[g, c]  = partition-group sums of W   (PE ones-matmuls, PSUM-accumulated,
              output already broadcast across all 128 partitions).
Engine balance (per-core busy estimate): DMA ~71us, ACT ~66 (2 big exps per
slot), DVE ~64 (recips + product in fp16 fast modes), Pool ~50 (13 of 17
A-reduces), PE ~15.
"""
import sys

import numpy as np

sys.path.insert(0, "/opt/trn_rl_repo")

from contextlib import ExitStack

import concourse.bass as bass  # noqa: F401  (registers AP machinery)
import concourse.tile as tile
from concourse import bacc, mybir
from concourse.bass_utils import run_bass_kernel_spmd

N = 8192          # full matrix side
B = 512           # block side
NB = N // B       # 16 blocks per side
P = 128           # SBUF partitions
T = B // P        # 4 row-subtiles per block
GRP = 256         # softmax group width
NG = B // GRP     # 2 groups per block side
TAU = 0.1
NSLOTS = 17       # block-pairs per core
NCORES = 8
YCLIP = -512.0    # lower clip for max-subtracted values (exp(10*y) == 0)

F16 = mybir.dt.float16
F32 = mybir.dt.float32
AF = mybir.ActivationFunctionType
OP = mybir.AluOpType

# Slots whose V-product + final multiply run on the Pool (gpsimd) engine
# instead of DVE. Empty: the neuronxcc walrus pipeline has no Pool-engine
# lowering for generic tensor ops (ISA check rejects them), so all product
# work stays on DVE.
POOL_PRODUCT_SLOTS = frozenset()


def core_pairs() -> list[list[tuple[int, int]]]:
    """136 upper-triangle block pairs distributed 17-per-core (2 diagonal
    pairs last per core; the device treats all slots uniformly)."""
    diag = [(i, i) for i in range(NB)]
    off = [(i, j) for i in range(NB) for j in range(i + 1, NB)]
    cps: list[list[tuple[int, int]]] = [[] for _ in range(NCORES)]
    for idx, p in enumerate(off):
        cps[idx % NCORES].append(p)
    for idx, p in enumerate(diag):
        cps[idx % NCORES].append(p)
    return cps


CORE_PAIRS = core_pairs()


def build():
    """Build + compile the (single-program, 8-core SPMD) Bass kernel."""
    nc = bacc.Bacc(
        "TRN2",
        target_bir_lowering=False,
        debug=False,
        enable_asserts=True,
        num_devices=NCORES,
    )
    a = nc.dram_tensor("a", [NSLOTS, P, T, B], F16, kind="ExternalInput").ap()
    w = nc.dram_tensor("w", [NSLOTS, P, T, B], F16, kind="ExternalInput").ap()
    o = nc.dram_tensor("o", [NSLOTS, P, T, B], F16, kind="ExternalOutput").ap()

    with tile.TileContext(nc) as tc, ExitStack() as ctx:
        const_pool = ctx.enter_context(tc.tile_pool(name="const", bufs=1))
        ones = const_pool.tile([P, P], F16)
        nc.vector.memset(ones[:], 1.0)

        a_pool = ctx.enter_context(tc.tile_pool(name="a_sb", bufs=3))
        w_pool = ctx.enter_context(tc.tile_pool(name="w_sb", bufs=3))
        za_pool = ctx.enter_context(tc.tile_pool(name="za", bufs=3))
        wz_pool = ctx.enter_context(tc.tile_pool(name="wz", bufs=3))
        v_pool = ctx.enter_context(tc.tile_pool(name="vv", bufs=2))
        o_pool = ctx.enter_context(tc.tile_pool(name="o_sb", bufs=3))
        st_pool = ctx.enter_context(tc.tile_pool(name="st", bufs=8))
        rb_pool = ctx.enter_context(tc.tile_pool(name="rb", bufs=2))
        ps_pool = ctx.enter_context(tc.tile_pool(name="ps", bufs=2, space="PSUM"))

        for k in range(NSLOTS):
            a_sb = a_pool.tile([P, T, B], F16)
            nc.sync.dma_start(a_sb[:], a[k])
            w_sb = w_pool.tile([P, T, B], F16)
            nc.sync.dma_start(w_sb[:], w[k])

            # Exponentials: one big-free-dim activation per side, fp16 out.
            za = za_pool.tile([P, T, B], F16)
            nc.scalar.activation(za[:], a_sb[:], AF.Exp, scale=1.0 / TAU)
            wz = wz_pool.tile([P, T, B], F16)
            nc.scalar.activation(wz[:], w_sb[:], AF.Exp, scale=1.0 / TAU)

            # B-side group sums on PE: sb_ps[:, g, c] = sum over the 256
            # B-columns of group g (= partitions of wz t-subtiles 2g, 2g+1),
            # broadcast to all 128 output partitions by the all-ones lhsT.
            sb_ps = ps_pool.tile([P, NG, B], F32)
            for t in range(T):
                nc.tensor.matmul(
                    sb_ps[:, t // 2, :], ones[:], wz[:, t, :],
                    start=(t % 2 == 0), stop=(t % 2 == 1),
                )

            # A-side group sums: two fp16 tree-add halvings at DVE 2x, then a
            # short f32 tensor_reduce (free-dim reduces are DVE-only, and
            # plain tensor_reduce has no fp16 fast mode -- the tree runs the
            # bulk of the work at 2x).
            za_g = za[:].rearrange("p t b -> p (t b)").rearrange(
                "p (G s) -> p G s", s=GRP
            )
            s1 = st_pool.tile([P, T * NG, GRP // 2], F16, name="s1")
            nc.vector.tensor_add(s1[:], za_g[:, :, 0:128], za_g[:, :, 128:256])
            s2 = st_pool.tile([P, T * NG, GRP // 4], F16, name="s2")
            nc.vector.tensor_add(s2[:], s1[:, :, 0:64], s1[:, :, 64:128])
            sa = st_pool.tile([P, T, NG], F32, name="sa")
            nc.vector.tensor_reduce(
                sa[:].rearrange("p t g -> p (t g)"),
                s2[:], axis=mybir.AxisListType.X, op=OP.add,
            )

            # 1/sB straight out of PSUM into fp16 SBUF (values in [1/256, 1]).
            rbinv = rb_pool.tile([P, NG, B], F16)
            with nc.allow_low_precision(reason="softmax recip scale, fp16 ok"):
                nc.vector.reciprocal(rbinv[:], sb_ps[:])

            ra = st_pool.tile([P, T, NG], F32, name="ra")
            nc.vector.reciprocal(ra[:], sa[:])

            # V = wz * rbinv (rbinv broadcast over the t-pair of each group):
            # all-fp16 SBUF operands -> DVE 2x mode. Then the final
            # out = (za * ra) * V as 8 scalar_tensor_tensor (fp16 SBUF ->
            # DVE 4x). For POOL_PRODUCT_SLOTS both stages run on the Pool
            # engine instead, trading cycle efficiency for DVE relief.
            eng = nc.gpsimd if k in POOL_PRODUCT_SLOTS else nc.vector
            v = v_pool.tile([P, T, B], F16)
            rb_b = rbinv[:].rearrange("p g (one c) -> p g one c", one=1) \
                .broadcast_to([P, NG, T // NG, B])
            eng.tensor_mul(
                v[:].rearrange("p (g u) b -> p g u b", g=NG), wz[:].rearrange(
                    "p (g u) b -> p g u b", g=NG), rb_b,
            )
            o_sb = o_pool.tile([P, T, B], F16)
            for t in range(T):
                for g in range(NG):
                    cs = slice(g * GRP, (g + 1) * GRP)
                    eng.scalar_tensor_tensor(
                        o_sb[:, t, cs], za[:, t, cs], ra[:, t, g:g + 1],
                        v[:, t, cs], op0=OP.mult, op1=OP.mult,
                    )
            # Stores ride the SWDGE (gpsimd) ring: they never queue ahead of
            # the sync-ring loads, and the Pool-engine dispatch cost is tiny.
            nc.gpsimd.dma_start(o[k], o_sb[:])

    nc.compile()
    return nc


_NC = None


def _get_nc():
    global _NC
    if _NC is None:
        _NC = build()
    return _NC


def _to_pmajor(blocks: np.ndarray) -> np.ndarray:
    # (n, 512, 512) row-major -> (n, 128, 4, 512): row r = t*P + p lands at
    # [p, t, :], so every SBUF partition's bytes are contiguous in DRAM.
    n = blocks.shape[0]
    return np.ascontiguousarray(
        blocks.reshape(n, T, P, B).transpose(0, 2, 1, 3)
    )


def _shifted_fp16(sims: np.ndarray) -> np.ndarray:
    """Y = sims - per-(row, 256-col-group) max, diagonal masked, clipped and
    cast to fp16. Softmax over any 256-aligned column group of Y matches the
    reference's (softmax shift invariance)."""
    y = np.array(sims, dtype=np.float32, copy=True)
    # Mask BEFORE the max: the group max must be over surviving entries,
    # else a dominant diagonal shifts the whole group into fp16 underflow
    # and the on-device group sum becomes 0 (-> inf/NaN).
    np.fill_diagonal(y, -np.inf)
    m = y.reshape(N, N // GRP, GRP).max(axis=-1, keepdims=True)
    y = (y.reshape(N, N // GRP, GRP) - m).reshape(N, N)
    np.clip(y, YCLIP, 0.0, out=y)
    return y.astype(np.float16)


def make_in_maps(sims: np.ndarray) -> list[dict[str, np.ndarray]]:
    yf = _shifted_fp16(sims)
    in_maps = []
    for c in range(NCORES):
        a_stack = np.empty((NSLOTS, B, B), np.float16)
        w_stack = np.empty((NSLOTS, B, B), np.float16)
        for k, (i, j) in enumerate(CORE_PAIRS[c]):
            a_stack[k] = yf[i * B:(i + 1) * B, j * B:(j + 1) * B]
            w_stack[k] = yf[j * B:(j + 1) * B, i * B:(i + 1) * B].T
        in_maps.append({"a": _to_pmajor(a_stack), "w": _to_pmajor(w_stack)})
    return in_maps


def assemble(results: list[dict[str, np.ndarray]]) -> np.ndarray:
    out = np.empty((N, N), np.float32)
    for c in range(NCORES):
        o_pm = results[c]["o"]  # (NSLOTS, P, T, B) fp16 partition-major
        o_stack = o_pm.transpose(0, 2, 1, 3).reshape(NSLOTS, B, B).astype(
            np.float32
        )
        for k, (i, j) in enumerate(CORE_PAIRS[c]):
            out[i * B:(i + 1) * B, j * B:(j + 1) * B] = o_stack[k]
            if i != j:
                out[j * B:(j + 1) * B, i * B:(i + 1) * B] = o_stack[k].T
    return out


def run_on_hw(sims: np.ndarray, **spmd_kwargs):
    """Run the kernel on the 8 NeuronCores. Returns (out, BassKernelResults).

    The device occasionally throws a transient NRT_EXEC_UNIT_UNRECOVERABLE
    and needs ~a minute to come back, so failed runs are retried."""
    import time

    nc = _get_nc()
    in_maps = make_in_maps(sims)
    last_exc = None
    for attempt in range(3):
        if attempt:
            time.sleep(75)
        try:
            res = run_bass_kernel_spmd(
                nc, in_maps, core_ids=list(range(NCORES)), **spmd_kwargs
            )
            return assemble(res.results), res
        except Exception as exc:  # noqa: BLE001 - device flake, retry
            last_exc = exc
    raise last_exc


def kernel(similarities: np.ndarray) -> np.ndarray:
    sims = np.ascontiguousarray(similarities, dtype=np.float32)
    assert sims.shape == (N, N)
    out, _ = run_on_hw(sims)
    return out


if __name__ == "__main__":
    rng = np.random.default_rng(0)
    sims = rng.standard_normal((N, N), dtype=np.float32)
    out = kernel(similarities=sims)
    print("out", out.shape, out.dtype, float(out.max()))


# revision 12
# speedup vs baseline: 1.4543x; 1.4543x over previous
"""Trainium2 Bass kernel for nn_BestHits: out = bh * bh.T where
bh = blockwise-softmax(mask_diag(similarities) / TAU) over 256-wide column groups.

Strategy: out is symmetric, so only the 136 upper-incl-diagonal 512x512
block-pairs are computed (17 per core on 8 cores); out[J,I] = out[I,J].T is
mirrored on the host.

fp16 end-to-end to halve HBM traffic (memory-bound problem): the host
subtracts the per-(row, 256-group) max before quantizing to fp16 (softmax is
shift-invariant, and the shift makes fp16 rounding error negligible: the
dominant softmax terms sit near 0 where fp16 absolute error is tiny).
Outputs are stored fp16 and widened to f32 on the host.

Per pair (I, J) the device sees A = Y[I, J]-block and W-input = Y[J, I].T
(transposed on host during staging, so the reciprocal product is purely
elementwise on device -- no PE transpose):
  out[I,J][r, c] = za[r, c]/sa[r, g(c)] * W[r, c]/sB[g(r), c]
with za = exp(10*A), W = exp(10*B.T),
  sa[r, g]  = row-group sums of za        (free-dim reduce: DVE/Pool),
  sB[g, c]  = partition-group sums of W   (PE ones-matmuls, PSUM-accumulated,
              output already broadcast across all 128 partitions).
Engine balance (per-core busy estimate): DMA ~71us, ACT ~66 (2 big exps per
slot), DVE ~64 (recips + product in fp16 fast modes), Pool ~50 (13 of 17
A-reduces), PE ~15.
"""
import sys

import numpy as np

sys.path.insert(0, "/opt/trn_rl_repo")

from contextlib import ExitStack

import concourse.bass as bass  # noqa: F401  (registers AP machinery)
import concourse.tile as tile
from concourse import bacc, mybir
from concourse.bass_utils import run_bass_kernel_spmd

N = 8192          # full matrix side
B = 512           # block side
NB = N // B       # 16 blocks per side
P = 128           # SBUF partitions
T = B // P        # 4 row-subtiles per block
GRP = 256         # softmax group width
NG = B // GRP     # 2 groups per block side
TAU = 0.1
NSLOTS = 17       # block-pairs per core
NCORES = 8
YCLIP = -512.0    # lower clip for max-subtracted values (exp(10*y) == 0)

F16 = mybir.dt.float16
F32 = mybir.dt.float32
AF = mybir.ActivationFunctionType
OP = mybir.AluOpType

# Slots whose V-product + final multiply run on the Pool (gpsimd) engine
# instead of DVE. Empty: the neuronxcc walrus pipeline has no Pool-engine
# lowering for generic tensor ops (ISA check rejects them), so all product
# work stays on DVE.
POOL_PRODUCT_SLOTS = frozenset()


def core_pairs() -> list[list[tuple[int, int]]]:
    """136 upper-triangle block pairs distributed 17-per-core (2 diagonal
    pairs last per core; the device treats all slots uniformly)."""
    diag = [(i, i) for i in range(NB)]
    off = [(i, j) for i in range(NB) for j in range(i + 1, NB)]
    cps: list[list[tuple[int, int]]] = [[] for _ in range(NCORES)]
    for idx, p in enumerate(off):
        cps[idx % NCORES].append(p)
    for idx, p in enumerate(diag):
        cps[idx % NCORES].append(p)
    return cps


CORE_PAIRS = core_pairs()


def build():
    """Build + compile the (single-program, 8-core SPMD) Bass kernel."""
    nc = bacc.Bacc(
        "TRN2",
        target_bir_lowering=False,
        debug=False,
        enable_asserts=True,
        num_devices=NCORES,
    )
    a = nc.dram_tensor("a", [NSLOTS, P, T, B], F16, kind="ExternalInput").ap()
    w = nc.dram_tensor("w", [NSLOTS, P, T, B], F16, kind="ExternalInput").ap()
    o = nc.dram_tensor("o", [NSLOTS, P, T, B], F16, kind="ExternalOutput").ap()

    with tile.TileContext(nc) as tc, ExitStack() as ctx:
        const_pool = ctx.enter_context(tc.tile_pool(name="const", bufs=1))
        ones = const_pool.tile([P, P], F16)
        nc.vector.memset(ones[:], 1.0)

        a_pool = ctx.enter_context(tc.tile_pool(name="a_sb", bufs=3))
        w_pool = ctx.enter_context(tc.tile_pool(name="w_sb", bufs=3))
        za_pool = ctx.enter_context(tc.tile_pool(name="za", bufs=3))
        wz_pool = ctx.enter_context(tc.tile_pool(name="wz", bufs=3))
        v_pool = ctx.enter_context(tc.tile_pool(name="vv", bufs=2))
        o_pool = ctx.enter_context(tc.tile_pool(name="o_sb", bufs=3))
        st_pool = ctx.enter_context(tc.tile_pool(name="st", bufs=8))
        rb_pool = ctx.enter_context(tc.tile_pool(name="rb", bufs=2))
        ps_pool = ctx.enter_context(tc.tile_pool(name="ps", bufs=2, space="PSUM"))

        for k in range(NSLOTS):
            a_sb = a_pool.tile([P, T, B], F16)
            nc.sync.dma_start(a_sb[:], a[k])
            w_sb = w_pool.tile([P, T, B], F16)
            nc.sync.dma_start(w_sb[:], w[k])

            # Exponentials: one big-free-dim activation per side, fp16 out.
            za = za_pool.tile([P, T, B], F16)
            nc.scalar.activation(za[:], a_sb[:], AF.Exp, scale=1.0 / TAU)
            wz = wz_pool.tile([P, T, B], F16)
            nc.scalar.activation(wz[:], w_sb[:], AF.Exp, scale=1.0 / TAU)

            # B-side group sums on PE: sb_ps[:, g, c] = sum over the 256
            # B-columns of group g (= partitions of wz t-subtiles 2g, 2g+1),
            # broadcast to all 128 output partitions by the all-ones lhsT.
            sb_ps = ps_pool.tile([P, NG, B], F32)
            for t in range(T):
                nc.tensor.matmul(
                    sb_ps[:, t // 2, :], ones[:], wz[:, t, :],
                    start=(t % 2 == 0), stop=(t % 2 == 1),
                )

            # A-side group sums: two fp16 tree-add halvings at DVE 2x, then a
            # short f32 tensor_reduce (free-dim reduces are DVE-only, and
            # plain tensor_reduce has no fp16 fast mode -- the tree runs the
            # bulk of the work at 2x).
            za_g = za[:].rearrange("p t b -> p (t b)").rearrange(
                "p (G s) -> p G s", s=GRP
            )
            s1 = st_pool.tile([P, T * NG, GRP // 2], F16, name="s1")
            nc.vector.tensor_add(s1[:], za_g[:, :, 0:128], za_g[:, :, 128:256])
            s2 = st_pool.tile([P, T * NG, GRP // 4], F16, name="s2")
            nc.vector.tensor_add(s2[:], s1[:, :, 0:64], s1[:, :, 64:128])
            s3 = st_pool.tile([P, T * NG, GRP // 8], F16, name="s3")
            nc.vector.tensor_add(s3[:], s2[:, :, 0:32], s2[:, :, 32:64])
            sa = st_pool.tile([P, T, NG], F32, name="sa")
            nc.vector.tensor_reduce(
                sa[:].rearrange("p t g -> p (t g)"),
                s3[:], axis=mybir.AxisListType.X, op=OP.add,
            )

            # Reciprocals via the ~5x-faster approx custom-DVE op (no divide
            # ALU op on DVE; the stock RECIPROCAL microcode measures
            # ~3.3us/instr). Inputs are group sums in [1, 256]: safely normal.
            rbinv = rb_pool.tile([P, NG, B], F32)
            nc.vector.reciprocal_approx_fast(rbinv[:], sb_ps[:])
            ra = st_pool.tile([P, T, NG], F32, name="ra")
            nc.vector.reciprocal_approx_fast(ra[:], sa[:])
            ra16 = st_pool.tile([P, T, NG], F16, name="ra16")
            nc.vector.tensor_copy(ra16[:], ra[:])

            # V = wz * rbinv (broadcast over the t-pair of each group).
            v = v_pool.tile([P, T, B], F16)
            rb_b = rbinv[:].rearrange("p g (one c) -> p g one c", one=1) \
                .broadcast_to([P, NG, T // NG, B])
            nc.vector.tensor_mul(
                v[:].rearrange("p (g u) b -> p g u b", g=NG), wz[:].rearrange(
                    "p (g u) b -> p g u b", g=NG), rb_b,
            )
            # out = (za * ra) * V: 8 scalar_tensor_tensor, all operands incl
            # the per-partition scalar in fp16 (fast-mode eligible).
            o_sb = o_pool.tile([P, T, B], F16)
            for t in range(T):
                for g in range(NG):
                    cs = slice(g * GRP, (g + 1) * GRP)
                    nc.vector.scalar_tensor_tensor(
                        o_sb[:, t, cs], za[:, t, cs], ra16[:, t, g:g + 1],
                        v[:, t, cs], op0=OP.mult, op1=OP.mult,
                    )
            # Stores ride the SWDGE (gpsimd) ring: they never queue ahead of
            # the sync-ring loads, and the Pool-engine dispatch cost is tiny.
            nc.gpsimd.dma_start(o[k], o_sb[:])

    nc.compile()
    return nc


_NC = None


def _get_nc():
    global _NC
    if _NC is None:
        _NC = build()
    return _NC


def _to_pmajor(blocks: np.ndarray) -> np.ndarray:
    # (n, 512, 512) row-major -> (n, 128, 4, 512): row r = t*P + p lands at
    # [p, t, :], so every SBUF partition's bytes are contiguous in DRAM.
    n = blocks.shape[0]
    return np.ascontiguousarray(
        blocks.reshape(n, T, P, B).transpose(0, 2, 1, 3)
    )


def _shifted_fp16(sims: np.ndarray) -> np.ndarray:
    """Y = sims - per-(row, 256-col-group) max, diagonal masked, clipped and
    cast to fp16. Softmax over any 256-aligned column group of Y matches the
    reference's (softmax shift invariance)."""
    y = np.array(sims, dtype=np.float32, copy=True)
    # Mask BEFORE the max: the group max must be over surviving entries,
    # else a dominant diagonal shifts the whole group into fp16 underflow
    # and the on-device group sum becomes 0 (-> inf/NaN).
    np.fill_diagonal(y, -np.inf)
    m = y.reshape(N, N // GRP, GRP).max(axis=-1, keepdims=True)
    y = (y.reshape(N, N // GRP, GRP) - m).reshape(N, N)
    np.clip(y, YCLIP, 0.0, out=y)
    return y.astype(np.float16)


def make_in_maps(sims: np.ndarray) -> list[dict[str, np.ndarray]]:
    yf = _shifted_fp16(sims)
    in_maps = []
    for c in range(NCORES):
        a_stack = np.empty((NSLOTS, B, B), np.float16)
        w_stack = np.empty((NSLOTS, B, B), np.float16)
        for k, (i, j) in enumerate(CORE_PAIRS[c]):
            a_stack[k] = yf[i * B:(i + 1) * B, j * B:(j + 1) * B]
            w_stack[k] = yf[j * B:(j + 1) * B, i * B:(i + 1) * B].T
        in_maps.append({"a": _to_pmajor(a_stack), "w": _to_pmajor(w_stack)})
    return in_maps


def assemble(results: list[dict[str, np.ndarray]]) -> np.ndarray:
    out = np.empty((N, N), np.float32)
    for c in range(NCORES):
        o_pm = results[c]["o"]  # (NSLOTS, P, T, B) fp16 partition-major
        o_stack = o_pm.transpose(0, 2, 1, 3).reshape(NSLOTS, B, B).astype(
            np.float32
        )
        for k, (i, j) in enumerate(CORE_PAIRS[c]):
            out[i * B:(i + 1) * B, j * B:(j + 1) * B] = o_stack[k]
            if i != j:
                out[j * B:(j + 1) * B, i * B:(i + 1) * B] = o_stack[k].T
    return out


def run_on_hw(sims: np.ndarray, **spmd_kwargs):
    """Run the kernel on the 8 NeuronCores. Returns (out, BassKernelResults).

    The device occasionally throws a transient NRT_EXEC_UNIT_UNRECOVERABLE
    and needs ~a minute to come back, so failed runs are retried."""
    import time

    nc = _get_nc()
    in_maps = make_in_maps(sims)
    last_exc = None
    for attempt in range(3):
        if attempt:
            time.sleep(75)
        try:
            res = run_bass_kernel_spmd(
                nc, in_maps, core_ids=list(range(NCORES)), **spmd_kwargs
            )
            return assemble(res.results), res
        except Exception as exc:  # noqa: BLE001 - device flake, retry
            last_exc = exc
    raise last_exc


def kernel(similarities: np.ndarray) -> np.ndarray:
    sims = np.ascontiguousarray(similarities, dtype=np.float32)
    assert sims.shape == (N, N)
    out, _ = run_on_hw(sims)
    return out


if __name__ == "__main__":
    rng = np.random.default_rng(0)
    sims = rng.standard_normal((N, N), dtype=np.float32)
    out = kernel(similarities=sims)
    print("out", out.shape, out.dtype, float(out.max()))
